# revision 2
# baseline (speedup 1.0000x reference)
"""Trainium2 Bass kernel for BertOutputWithAdapterFusion (T=8 adapters).

Math (reference):
  h        = hidden_states @ dense_w.T + dense_b          [B,S,H]
  prenorm  = input_tensor + h
  ain      = LN(prenorm)
  down[t]  = relu(ain @ down_w[t].T + down_b[t])          [B,S,D]
  aout[t]  = down[t] @ up_w[t].T + up_b[t]                [B,S,H]
  key[t]   = aout[t] @ key_w.T + key_b
  value[t] = (aout[t] + h) @ value_w.T
  query    = prenorm @ query_w.T + query_b
  scores   = sum(query*key, -1); probs = softmax_t(scores)
  fusion   = sum_t probs[t]*value[t]
  out      = LN(input_tensor + fusion)

Algebraic restructuring used here (exact, up to fp rounding):
  qk       = query @ key_w                   (scores[t] = aout[t]·qk + q·key_b,
                                              and the q·key_b term is constant
                                              over t so it cancels in softmax)
  scores[t]= sum_d down[t]*qkproj[t] + up_b[t]·qk
             where qkproj[t] = qk @ up_w[t]  (aout never materialized)
  fusion   = (mix + h) @ value_w.T
             where mix = sum_t probs[t]*aout[t]
                       = up_w-combine(probs[t]*down[t]) + probs.T @ up_b
                       (uses sum_t probs[t] = 1)
The T=8 adapters are stacked into single [H, T*D] matmuls.

Distribution: data-parallel over the 8 batches (1 batch of 512 rows per
core), all parameters replicated, no collectives.  All on-chip activations
are kept transposed [feature, row] so no on-chip transposes are needed —
every weight is pre-transposed/packed on the host.  Matmuls run as fp32r
(full PE rate at N=512); LayerNorm stats and T-axis softmax reductions run
as ones-vector matmuls over the partition axis.
"""
import sys
import types

sys.path.insert(0, "/opt/trn_rl_repo")

import numpy as np
from contextlib import ExitStack

import concourse.bacc as bacc
import concourse.tile as tile
from concourse import mybir
import concourse.bass_utils as bass_utils
from concourse.bass_utils import run_bass_kernel_spmd

# ── axon NTFF profile hook (image's antenv lacks axon_hooks) ─────────────
try:
    from trn_agent_boot.trn_boot import _ntff_profile_via_ctypes

    _hook = _ntff_profile_via_ctypes("/opt/axon/libaxon_pjrt.so")
except Exception:  # pragma: no cover
    _hook = None
_mod = types.ModuleType("antenv.axon_hooks")
_mod.get_axon_ntff_profile_hook = lambda: _hook
_mod.set_axon_ntff_profile_hook = lambda h: None
sys.modules["antenv.axon_hooks"] = _mod
bass_utils.upload_artifacts = lambda tmpdir: str(tmpdir)

# ── problem shapes (hardcoded per spec) ──────────────────────────────────
B, S, I, H, T, D = 8, 512, 4096, 1024, 8, 64
NCORES = 8
R = (B * S) // NCORES        # 512 rows per core
P = 128
IC, HC = I // P, H // P      # 32, 8
TD = T * D                   # 512
TDC = TD // P                # 4
EPS = 1e-12

F32 = mybir.dt.float32
F32R = mybir.dt.float32r
AF = mybir.ActivationFunctionType
ALU = mybir.AluOpType


def _pack_k(w):
    """[K, M] -> [P, (K//P)*M] so that slab kc (cols kc*M:(kc+1)*M) is the
    [P, M] tile holding rows kc*P..kc*P+P of w, partition-major."""
    K, M = w.shape
    return np.ascontiguousarray(
        w.reshape(K // P, P, M).transpose(1, 0, 2).reshape(P, (K // P) * M)
    )


def build_kernel():
    nc = bacc.Bacc("TRN2", debug=False)

    dram = lambda name, shape: nc.dram_tensor(name, shape, F32, kind="ExternalInput")
    # per-core activations
    xt_p = dram("xt_p", [P, IC * R])       # packed X^T slice
    inpt_p = dram("inpt_p", [P, HC * R])   # packed input_tensor^T slice
    # packed weights (replicated)
    wdt_p = dram("wdt_p", [P, IC * H])     # dense_w.T packed
    qwt_p = dram("qwt_p", [P, HC * H])     # query_w.T packed
    kw_p = dram("kw_p", [P, HC * H])       # key_w packed
    vwt_p = dram("vwt_p", [P, HC * H])     # value_w.T packed
    dwt_p = dram("dwt_p", [P, HC * TD])    # down_w 't d h -> h (t d)' packed
    uw1_p = dram("uw1_p", [P, HC * TD])    # up_w 't h d -> h (t d)' packed
    uw2_p = dram("uw2_p", [P, TDC * H])    # up_w 't h d -> (t d) h' packed
    upb = dram("upb", [T, H])              # up_b
    upbt_p = dram("upbt_p", [P, HC * T])   # up_b.T packed
    dben = dram("dben", [1, H])            # dense_b row
    qb_c = dram("qb_c", [P, HC])           # query_b per-chunk cols
    db_c = dram("db_c", [P, TDC])          # down_b per-chunk cols
    lng_c = dram("lng_c", [P, HC])         # ln_g per-chunk cols
    lnb_c = dram("lnb_c", [P, HC])         # ln_b per-chunk cols
    ones_r = dram("ones_r", [1, R])        # ones row
    ones_p = dram("ones_p", [P, 1])        # ones column
    sel_p = dram("sel_p", [P, TDC * T])    # per-chunk task-sum selectors
    exm_p = dram("exm_p", [T, TDC * P])    # per-chunk task-broadcast expanders
    out_p = nc.dram_tensor("out_p", [P, HC * R], F32, kind="ExternalOutput")

    with tile.TileContext(nc) as tc:
        with ExitStack() as ctx:
            const = ctx.enter_context(tc.tile_pool(name="const", bufs=1))
            acts = ctx.enter_context(tc.tile_pool(name="acts", bufs=1))
            wp = ctx.enter_context(tc.tile_pool(name="wp", bufs=1))

            def cdma(shape, dt, src):
                t = const.tile(shape, dt, name=f"c_{src.tensor.name}", uniquify=True)
                nc.sync.dma_start(out=t, in_=src if dt is F32 else src.bitcast(F32R))
                return t

            ones_r_sb = cdma([1, R], F32R, ones_r[:, :])
            ones_p_sb = cdma([P, 1], F32R, ones_p[:, :])
            dben_sb = cdma([1, H], F32R, dben[:, :])
            upb_sb = cdma([T, H], F32R, upb[:, :])
            sel_sb = cdma([P, TDC * T], F32R, sel_p[:, :])
            exm_sb = cdma([T, TDC * P], F32R, exm_p[:, :])
            upbt_sb = cdma([P, HC * T], F32R, upbt_p[:, :])
            qb_sb = cdma([P, HC], F32, qb_c[:, :])
            db_sb = cdma([P, TDC], F32, db_c[:, :])
            lng_sb = cdma([P, HC], F32, lng_c[:, :])
            lnb_sb = cdma([P, HC], F32, lnb_c[:, :])
            eps_sb = const.tile([1, 1], F32)
            nc.vector.memset(eps_sb, EPS)

            # input_tensor^T chunks (f32, element-wise use only)
            inpt = []
            for m in range(HC):
                it = acts.tile([P, R], F32, tag="u", bufs=44, name=f"inpt{m}")
                nc.sync.dma_start(out=it, in_=inpt_p[:, m * R:(m + 1) * R])
                inpt.append(it)

            def atile(name, dt=F32R):
                return acts.tile([P, R], dt, tag="u", bufs=44, name=name)

            def wtile_a(name):
                return wp.tile([P, H], F32R, tag="wa", bufs=3, name=name)

            def wtile5(name):
                return wp.tile([P, TD], F32R, tag="w5", bufs=18, name=name)

            # ═══ Phase A: h^T = dense_w.T.T @ X^T (+ dense_b) ═══
            ht = []       # h^T chunks, f32 (element-wise use)
            prenorm = []  # prenorm^T chunks, f32r
            with tc.tile_pool(name="psA", bufs=1, space="PSUM") as psA:
                psa = [psA.tile([P, R], F32, tag="pa", bufs=8, name=f"psa{m}")
                       for m in range(HC)]
                # rank-1 dense_b via K=1 matmul
                for m in range(HC):
                    nc.tensor.matmul(psa[m][:], dben_sb[:, m * P:(m + 1) * P],
                                     ones_r_sb[:], start=True, stop=False)
                for kc in range(IC):
                    wsl = wtile_a(f"wa{kc}")
                    nc.sync.dma_start(out=wsl, in_=wdt_p[:, kc * H:(kc + 1) * H]
                                      .bitcast(F32R))
                    xsl = wp.tile([P, R], F32R, tag="xt", bufs=3, name=f"xt{kc}")
                    nc.sync.dma_start(out=xsl, in_=xt_p[:, kc * R:(kc + 1) * R]
                                      .bitcast(F32R))
                    for m in range(HC):
                        nc.tensor.matmul(psa[m][:], wsl[:, m * P:(m + 1) * P],
                                         xsl[:], start=False, stop=(kc == IC - 1))
                for m in range(HC):
                    h_m = atile(f"ht{m}", F32)
                    nc.scalar.copy(out=h_m[:], in_=psa[m][:])
                    ht.append(h_m)
                    pn = atile(f"prenorm{m}")
                    nc.vector.tensor_tensor(out=pn[:], in0=psa[m][:],
                                            in1=inpt[m][:], op=ALU.add)
                    prenorm.append(pn)

            psM = ctx.enter_context(tc.tile_pool(name="psM", bufs=1, space="PSUM"))
            psX = ctx.enter_context(tc.tile_pool(name="psX", bufs=1, space="PSUM"))
            psS = ctx.enter_context(tc.tile_pool(name="psS", bufs=1, space="PSUM"))

            def pmain(name):
                return psM.tile([P, R], F32, tag="pm", bufs=4, name=name)

            def paux(name):
                return psX.tile([P, R], F32, tag="px", bufs=2, name=name)

            def psmall(name):
                return psS.tile([T, R], F32, tag="ps", bufs=2, name=name)

            # ── LayerNorm helpers (stats over the partition=feature axis) ──
            def ln_stats(chunks, label):
                """-> (mu_b, rstd_b) PSUM [P,R] broadcasts; emits sum MMs now."""
                ssum = psmall(f"{label}_sum")
                ssq = psmall(f"{label}_ssq")
                sqs = []
                for m in range(HC):
                    sq = acts.tile([P, R], F32R, tag="sq", bufs=3,
                                   name=f"{label}_sq{m}")
                    nc.scalar.square(out=sq[:], in_=chunks[m][:].bitcast(F32))
                    sqs.append(sq)
                for m in range(HC):
                    nc.tensor.matmul(ssum[:1, :], ones_p_sb[:], chunks[m][:],
                                     start=(m == 0), stop=(m == HC - 1))
                    nc.tensor.matmul(ssq[:1, :], ones_p_sb[:], sqs[m][:],
                                     start=(m == 0), stop=(m == HC - 1))
                return ssum, ssq

            def ln_finish(ssum, ssq, label):
                """Small-op chain + broadcast matmuls."""
                mu = acts.tile([1, R], F32R, tag="st", bufs=4, name=f"{label}_mu")
                nc.scalar.mul(out=mu[:], in_=ssum[:1, :], mul=1.0 / H)
                msq = acts.tile([1, R], F32, tag="st", bufs=4, name=f"{label}_msq")
                nc.scalar.mul(out=msq[:], in_=ssq[:1, :], mul=1.0 / H)
                musq = acts.tile([1, R], F32, tag="st", bufs=4, name=f"{label}_musq")
                nc.scalar.square(out=musq[:], in_=mu[:].bitcast(F32))
                var = acts.tile([1, R], F32, tag="st", bufs=4, name=f"{label}_var")
                nc.vector.tensor_tensor(out=var[:], in0=msq[:], in1=musq[:],
                                        op=ALU.subtract)
                sd = acts.tile([1, R], F32, tag="st", bufs=4, name=f"{label}_sd")
                nc.scalar.activation(out=sd[:], in_=var[:], func=AF.Sqrt,
                                     bias=eps_sb[:])
                rstd = acts.tile([1, R], F32R, tag="st", bufs=4, name=f"{label}_rstd")
                with nc.allow_low_precision(reason="f32r feed for broadcast mm"):
                    nc.vector.reciprocal(out=rstd[:], in_=sd[:])
                mu_b = paux(f"{label}_mub")
                nc.tensor.matmul(mu_b[:], ones_r_sb[:, :P], mu[:],
                                 start=True, stop=True)
                rstd_b = paux(f"{label}_rstdb")
                nc.tensor.matmul(rstd_b[:], ones_r_sb[:, :P], rstd[:],
                                 start=True, stop=True)
                return mu_b, rstd_b

            def ln_normalize(chunks, mu_b, rstd_b, g_sb, b_sb, label,
                             out_dt, out_names=None):
                outs = []
                for m in range(HC):
                    t1 = acts.tile([P, R], F32, tag="t1", bufs=3,
                                   name=f"{label}_t1_{m}")
                    nc.vector.tensor_tensor(out=t1[:], in0=chunks[m][:].bitcast(F32),
                                            in1=mu_b[:], op=ALU.subtract)
                    t2 = acts.tile([P, R], F32, tag="t2", bufs=3,
                                   name=f"{label}_t2_{m}")
                    nc.vector.tensor_tensor(out=t2[:], in0=t1[:], in1=rstd_b[:],
                                            op=ALU.mult)
                    name = out_names[m] if out_names else f"{label}_o{m}"
                    o = atile(name, out_dt) if out_names is None else \
                        acts.tile([P, R], out_dt, tag="ov", bufs=3, name=name)
                    nc.scalar.activation(out=o[:], in_=t2[:], func=AF.Identity,
                                         scale=g_sb[:, m:m + 1],
                                         bias=b_sb[:, m:m + 1])
                    outs.append(o)
                return outs

            # ═══ LN1 stats (sums on PE now; finish after C/D) ═══
            ln1_sum, ln1_ssq = ln_stats(prenorm, "ln1")

            # ═══ Phase C: q^T = query_w.T.T @ prenorm^T + query_b ═══
            qw_sl = [wtile5(f"wq{i}") for i in range(2 * HC)]
            for i in range(2 * HC):
                nc.sync.dma_start(out=qw_sl[i], in_=qwt_p[:, i * TD:(i + 1) * TD]
                                  .bitcast(F32R))
            q = []
            for m in range(HC):
                ps = pmain(f"psc{m}")
                for kc in range(HC):
                    nc.tensor.matmul(
                        ps[:],
                        qw_sl[kc * 2 + m // 4][:, (m % 4) * P:(m % 4 + 1) * P],
                        prenorm[kc][:], start=(kc == 0),
                        stop=(kc == HC - 1))
                qm = atile(f"q{m}")
                nc.scalar.activation(out=qm[:], in_=ps[:], func=AF.Identity,
                                     bias=qb_sb[:, m:m + 1])
                q.append(qm)

            # ═══ Phase D: qk^T = key_w.T @ q^T ═══
            kw_sl = [wtile5(f"wk{i}") for i in range(2 * HC)]
            for i in range(2 * HC):
                nc.sync.dma_start(out=kw_sl[i], in_=kw_p[:, i * TD:(i + 1) * TD]
                                  .bitcast(F32R))
            qk = []
            for m in range(HC):
                ps = pmain(f"psd{m}")
                for kc in range(HC):
                    nc.tensor.matmul(
                        ps[:],
                        kw_sl[kc * 2 + m // 4][:, (m % 4) * P:(m % 4 + 1) * P],
                        q[kc][:], start=(kc == 0), stop=(kc == HC - 1))
                qkm = atile(f"qk{m}")
                nc.scalar.copy(out=qkm[:], in_=ps[:])
                qk.append(qkm)

            # ═══ LN1 finish + normalize -> adapter_in^T (f32r) ═══
            mu_b, rstd_b = ln_finish(ln1_sum, ln1_ssq, "ln1")

            # ═══ Phase E: qkproj^T = up_w'(h,(t d)).T @ qk^T ═══
            u1_sl = [wtile5(f"wu1{kc}") for kc in range(HC)]
            for kc in range(HC):
                nc.sync.dma_start(out=u1_sl[kc], in_=uw1_p[:, kc * TD:(kc + 1) * TD]
                                  .bitcast(F32R))
            qkproj = []
            for c in range(TDC):
                ps = pmain(f"pse{c}")
                for kc in range(HC):
                    nc.tensor.matmul(ps[:], u1_sl[kc][:, c * P:(c + 1) * P],
                                     qk[kc][:], start=(kc == 0), stop=(kc == HC - 1))
                qp = atile(f"qkproj{c}", F32)
                nc.scalar.copy(out=qp[:], in_=ps[:])
                qkproj.append(qp)

            ain = ln_normalize(prenorm, mu_b, rstd_b, lng_sb, lnb_sb, "ln1", F32R)

            # ═══ Phase B: down^T = relu(down_w'(h,(t d)).T @ ain^T + down_b) ═══
            dw_sl = [wtile5(f"wd{kc}") for kc in range(HC)]
            for kc in range(HC):
                nc.sync.dma_start(out=dw_sl[kc], in_=dwt_p[:, kc * TD:(kc + 1) * TD]
                                  .bitcast(F32R))
            down = []
            for c in range(TDC):
                ps = pmain(f"psb{c}")
                for kc in range(HC):
                    nc.tensor.matmul(ps[:], dw_sl[kc][:, c * P:(c + 1) * P],
                                     ain[kc][:], start=(kc == 0), stop=(kc == HC - 1))
                dn = atile(f"down{c}")
                nc.scalar.activation(out=dn[:], in_=ps[:], func=AF.Relu,
                                     bias=db_sb[:, c:c + 1])
                down.append(dn)

            # ═══ scores = sum_d down*qkproj + up_b·qk  (PSUM [T, R]) ═══
            scores = psmall("scores")
            for kc in range(HC):
                nc.tensor.matmul(scores[:], upbt_sb[:, kc * T:(kc + 1) * T],
                                 qk[kc][:], start=(kc == 0), stop=False)
            prods = []
            for c in range(TDC):
                pr = atile(f"prod{c}")
                nc.vector.tensor_tensor(out=pr[:], in0=down[c][:].bitcast(F32),
                                        in1=qkproj[c][:], op=ALU.mult)
                prods.append(pr)
            for c in range(TDC):
                nc.tensor.matmul(scores[:], sel_sb[:, c * T:(c + 1) * T],
                                 prods[c][:], start=False, stop=(c == TDC - 1))

            # ═══ softmax over T (partition axis, 8 rows) ═══
            exp_sb = acts.tile([T, R], F32R, tag="s8", bufs=2, name="exp_sb")
            nc.scalar.activation(out=exp_sb[:], in_=scores[:], func=AF.Exp)
            sumexp = psmall("sumexp")
            nc.tensor.matmul(sumexp[:1, :], ones_p_sb[:T, :], exp_sb[:],
                             start=True, stop=True)
            rec = acts.tile([1, R], F32R, tag="st", bufs=4, name="rec")
            with nc.allow_low_precision(reason="softmax recip feeds broadcast mm"):
                nc.vector.reciprocal(out=rec[:], in_=sumexp[:1, :])
            rec8 = psmall("rec8")
            nc.tensor.matmul(rec8[:], ones_r_sb[:, :T], rec[:], start=True, stop=True)
            probs = acts.tile([T, R], F32R, tag="s8", bufs=2, name="probs")
            nc.vector.tensor_tensor(out=probs[:], in0=exp_sb[:].bitcast(F32),
                                    in1=rec8[:], op=ALU.mult)

            # wdown = probs-broadcast * down
            wdown = []
            for c in range(TDC):
                pb = paux(f"pbx{c}")
                nc.tensor.matmul(pb[:], exm_sb[:, c * P:(c + 1) * P], probs[:],
                                 start=True, stop=True)
                wd = atile(f"wdown{c}")
                nc.vector.tensor_tensor(out=wd[:], in0=down[c][:].bitcast(F32),
                                        in1=pb[:], op=ALU.mult)
                wdown.append(wd)

            # ═══ Phase F: mix^T = up_w''((t d),h).T @ wdown + up_b.T @ probs ═══
            u2_sl = [wtile5(f"wu2{i}") for i in range(2 * TDC)]
            for i in range(2 * TDC):
                nc.sync.dma_start(out=u2_sl[i], in_=uw2_p[:, i * TD:(i + 1) * TD]
                                  .bitcast(F32R))
            mixh = []
            for m in range(HC):
                ps = pmain(f"psf{m}")
                nc.tensor.matmul(ps[:], upb_sb[:, m * P:(m + 1) * P], probs[:],
                                 start=True, stop=False)
                for kc in range(TDC):
                    nc.tensor.matmul(
                        ps[:],
                        u2_sl[kc * 2 + m // 4][:, (m % 4) * P:(m % 4 + 1) * P],
                        wdown[kc][:], start=False, stop=(kc == TDC - 1))
                mh = atile(f"mixh{m}")
                nc.vector.tensor_tensor(out=mh[:], in0=ps[:], in1=ht[m][:],
                                        op=ALU.add)
                mixh.append(mh)

            # ═══ Phase G: fusion^T = value_w.T.T @ (mix+h)^T;  pre2 = input + fusion ═══
            vw_sl = [wtile5(f"wv{i}") for i in range(2 * HC)]
            for i in range(2 * HC):
                nc.sync.dma_start(out=vw_sl[i], in_=vwt_p[:, i * TD:(i + 1) * TD]
                                  .bitcast(F32R))
            inpt2 = []
            for m in range(HC):
                it2 = atile(f"inpt2_{m}", F32)
                nc.sync.dma_start(out=it2, in_=inpt_p[:, m * R:(m + 1) * R])
                inpt2.append(it2)
            pre2 = []
            for m in range(HC):
                ps = pmain(f"psg{m}")
                for kc in range(HC):
                    nc.tensor.matmul(
                        ps[:],
                        vw_sl[kc * 2 + m // 4][:, (m % 4) * P:(m % 4 + 1) * P],
                        mixh[kc][:], start=(kc == 0), stop=(kc == HC - 1))
                p2 = atile(f"pre2_{m}")
                nc.vector.tensor_tensor(out=p2[:], in0=ps[:], in1=inpt2[m][:],
                                        op=ALU.add)
                pre2.append(p2)

            # ═══ LN2 -> out ═══
            ln2_sum, ln2_ssq = ln_stats(pre2, "ln2")
            mu2_b, rstd2_b = ln_finish(ln2_sum, ln2_ssq, "ln2")
            outs = ln_normalize(pre2, mu2_b, rstd2_b, lng_sb, lnb_sb, "ln2", F32,
                                out_names=[f"outv{m}" for m in range(HC)])
            for m in range(HC):
                nc.sync.dma_start(out=out_p[:, m * R:(m + 1) * R], in_=outs[m][:])

    nc.compile()
    return nc


_NC_CACHE = None


def _get_nc():
    global _NC_CACHE
    if _NC_CACHE is None:
        _NC_CACHE = build_kernel()
    return _NC_CACHE


def _prep_weights(dense_w, dense_b, ln_g, ln_b, down_w, down_b, up_w, up_b,
                  key_w, key_b, query_w, query_b, value_w):
    f = np.float32
    sel = np.zeros((P, TDC * T), f)
    exm = np.zeros((T, TDC * P), f)
    for c in range(TDC):
        for k in range(P):
            t = c * 2 + (k // 64)
            sel[k, c * T + t] = 1.0
            exm[t, c * P + k] = 1.0
    return {
        "wdt_p": _pack_k(np.ascontiguousarray(dense_w.T).astype(f)),
        "qwt_p": _pack_k(np.ascontiguousarray(query_w.T).astype(f)),
        "kw_p": _pack_k(np.ascontiguousarray(key_w).astype(f)),
        "vwt_p": _pack_k(np.ascontiguousarray(value_w.T).astype(f)),
        "dwt_p": _pack_k(np.ascontiguousarray(
            down_w.transpose(2, 0, 1).reshape(H, TD)).astype(f)),
        "uw1_p": _pack_k(np.ascontiguousarray(
            up_w.transpose(1, 0, 2).reshape(H, TD)).astype(f)),
        "uw2_p": _pack_k(np.ascontiguousarray(
            up_w.transpose(0, 2, 1).reshape(TD, H)).astype(f)),
        "upb": up_b.astype(f),
        "upbt_p": _pack_k(np.ascontiguousarray(up_b.T).astype(f)),
        "dben": dense_b.reshape(1, H).astype(f),
        "qb_c": np.ascontiguousarray(query_b.reshape(HC, P).T).astype(f),
        "db_c": np.ascontiguousarray(down_b.reshape(TD).reshape(TDC, P).T).astype(f),
        "lng_c": np.ascontiguousarray(ln_g.reshape(HC, P).T).astype(f),
        "lnb_c": np.ascontiguousarray(ln_b.reshape(HC, P).T).astype(f),
        "ones_r": np.ones((1, R), f),
        "ones_p": np.ones((P, 1), f),
        "sel_p": sel,
        "exm_p": exm,
    }


def kernel(hidden_states, input_tensor, dense_w, dense_b, ln_g, ln_b,
           down_w, down_b, up_w, up_b, key_w, key_b, query_w, query_b,
           value_w, _trace=False):
    nc = _get_nc()
    hidden_states = np.asarray(hidden_states, np.float32)
    input_tensor = np.asarray(input_tensor, np.float32)
    wmap = _prep_weights(np.asarray(dense_w), np.asarray(dense_b),
                         np.asarray(ln_g), np.asarray(ln_b),
                         np.asarray(down_w), np.asarray(down_b),
                         np.asarray(up_w), np.asarray(up_b),
                         np.asarray(key_w), np.asarray(key_b),
                         np.asarray(query_w), np.asarray(query_b),
                         np.asarray(value_w))
    # key_b only shifts all task scores equally -> cancels in softmax_t.
    xt = np.ascontiguousarray(hidden_states.reshape(B * S, I).T)   # [I, B*S]
    it = np.ascontiguousarray(input_tensor.reshape(B * S, H).T)    # [H, B*S]
    in_maps = []
    for c in range(NCORES):
        m = dict(wmap)
        m["xt_p"] = _pack_k(np.ascontiguousarray(xt[:, c * R:(c + 1) * R]))
        m["inpt_p"] = _pack_k(np.ascontiguousarray(it[:, c * R:(c + 1) * R]))
        in_maps.append(m)

    res = run_bass_kernel_spmd(nc, in_maps, core_ids=list(range(NCORES)),
                               trace=_trace)
    out = np.empty((B * S, H), np.float32)
    for c in range(NCORES):
        op = res.results[c]["out_p"]
        oc = op.reshape(P, HC, R).transpose(1, 0, 2).reshape(H, R)
        out[c * R:(c + 1) * R, :] = oc.T
    out = out.reshape(B, S, H)
    if _trace:
        return out, res
    return out


# revision 4
# speedup vs baseline: 1.2884x; 1.2884x over previous
"""Trainium2 Bass kernel for BertOutputWithAdapterFusion (T=8 adapters).

Math (reference):
  h        = hidden_states @ dense_w.T + dense_b          [B,S,H]
  prenorm  = input_tensor + h
  ain      = LN(prenorm)
  down[t]  = relu(ain @ down_w[t].T + down_b[t])          [B,S,D]
  aout[t]  = down[t] @ up_w[t].T + up_b[t]                [B,S,H]
  key[t]   = aout[t] @ key_w.T + key_b
  value[t] = (aout[t] + h) @ value_w.T
  query    = prenorm @ query_w.T + query_b
  scores   = sum(query*key, -1); probs = softmax_t(scores)
  fusion   = sum_t probs[t]*value[t]
  out      = LN(input_tensor + fusion)

Algebraic restructuring used here (exact, up to fp rounding):
  qk       = query @ key_w                   (scores[t] = aout[t]·qk + q·key_b,
                                              and the q·key_b term is constant
                                              over t so it cancels in softmax)
  scores[t]= sum_d down[t]*qkproj[t] + up_b[t]·qk
             where qkproj[t] = qk @ up_w[t]  (aout never materialized)
  fusion   = (mix + h) @ value_w.T
             where mix = sum_t probs[t]*aout[t]
                       = up_w-combine(probs[t]*down[t]) + probs.T @ up_b
                       (uses sum_t probs[t] = 1)
The T=8 adapters are stacked into single [H, T*D] matmuls.

Distribution: data-parallel over the 8 batches (1 batch of 512 rows per
core), all parameters replicated, no collectives.  All on-chip activations
are kept transposed [feature, row] so no on-chip transposes are needed —
every weight is pre-transposed/packed on the host.  Matmuls run as fp32r
(full PE rate at N=512); LayerNorm stats and T-axis softmax reductions run
as ones-vector matmuls over the partition axis.
"""
import sys
import types

sys.path.insert(0, "/opt/trn_rl_repo")

import numpy as np
import ml_dtypes
from contextlib import ExitStack

import concourse.bacc as bacc
import concourse.tile as tile
from concourse import mybir
import concourse.bass_utils as bass_utils
from concourse.bass_utils import run_bass_kernel_spmd

# ── axon NTFF profile hook (image's antenv lacks axon_hooks) ─────────────
try:
    from trn_agent_boot.trn_boot import _ntff_profile_via_ctypes

    _hook = _ntff_profile_via_ctypes("/opt/axon/libaxon_pjrt.so")
except Exception:  # pragma: no cover
    _hook = None
_mod = types.ModuleType("antenv.axon_hooks")
_mod.get_axon_ntff_profile_hook = lambda: _hook
_mod.set_axon_ntff_profile_hook = lambda h: None
sys.modules["antenv.axon_hooks"] = _mod
bass_utils.upload_artifacts = lambda tmpdir: str(tmpdir)

# ── problem shapes (hardcoded per spec) ──────────────────────────────────
B, S, I, H, T, D = 8, 512, 4096, 1024, 8, 64
NCORES = 8
R = (B * S) // NCORES        # 512 rows per core
P = 128
IC, HC = I // P, H // P      # 32, 8
TD = T * D                   # 512
TDC = TD // P                # 4
EPS = 1e-12

F32 = mybir.dt.float32
F32R = mybir.dt.float32r
BF16 = mybir.dt.bfloat16
AF = mybir.ActivationFunctionType
ALU = mybir.AluOpType


def _pack_k(w):
    """[K, M] -> [P, (K//P)*M] so that slab kc (cols kc*M:(kc+1)*M) is the
    [P, M] tile holding rows kc*P..kc*P+P of w, partition-major."""
    K, M = w.shape
    return np.ascontiguousarray(
        w.reshape(K // P, P, M).transpose(1, 0, 2).reshape(P, (K // P) * M)
    )


def build_kernel():
    nc = bacc.Bacc("TRN2", debug=False)

    dram = lambda name, shape: nc.dram_tensor(name, shape, F32, kind="ExternalInput")
    # per-core activations
    xt_p = nc.dram_tensor("xt_p", [P, IC * R], BF16, kind="ExternalInput")
    inpt_p = dram("inpt_p", [P, HC * R])   # packed input_tensor^T slice
    # packed weights (replicated)
    wdt_p = nc.dram_tensor("wdt_p", [P, IC * H], BF16, kind="ExternalInput")
    qwt_p = dram("qwt_p", [P, HC * H])     # query_w.T packed
    kw_p = dram("kw_p", [P, HC * H])       # key_w packed
    vwt_p = dram("vwt_p", [P, HC * H])     # value_w.T packed
    dwt_p = dram("dwt_p", [P, HC * TD])    # down_w 't d h -> h (t d)' packed
    uw1_p = dram("uw1_p", [P, HC * TD])    # up_w 't h d -> h (t d)' packed
    uw2_p = dram("uw2_p", [P, TDC * H])    # up_w 't h d -> (t d) h' packed
    upb = dram("upb", [T, H])              # up_b
    upbt_p = dram("upbt_p", [P, HC * T])   # up_b.T packed
    dben = dram("dben", [1, H])            # dense_b row
    qb_c = dram("qb_c", [P, HC])           # query_b per-chunk cols
    db_c = dram("db_c", [P, TDC])          # down_b per-chunk cols
    lng_c = dram("lng_c", [P, HC])         # ln_g per-chunk cols
    lnb_c = dram("lnb_c", [P, HC])         # ln_b per-chunk cols
    ones_r = dram("ones_r", [1, R])        # ones row
    ones_p = dram("ones_p", [P, 1])        # ones column
    sel_p = dram("sel_p", [P, TDC * T])    # per-chunk task-sum selectors
    exm_p = dram("exm_p", [T, TDC * P])    # per-chunk task-broadcast expanders
    out_p = nc.dram_tensor("out_p", [P, HC * R], F32, kind="ExternalOutput")

    with tile.TileContext(nc) as tc:
        with ExitStack() as ctx:
            const = ctx.enter_context(tc.tile_pool(name="const", bufs=1))
            acts = ctx.enter_context(tc.tile_pool(name="acts", bufs=1))
            wp = ctx.enter_context(tc.tile_pool(name="wp", bufs=1))

            def cdma(shape, dt, src):
                t = const.tile(shape, dt, name=f"c_{src.tensor.name}", uniquify=True)
                nc.sync.dma_start(out=t, in_=src if dt is F32 else src.bitcast(F32R))
                return t

            ones_r_sb = cdma([1, R], F32R, ones_r[:, :])
            ones_p_sb = cdma([P, 1], F32R, ones_p[:, :])
            dben_sb = cdma([1, H], F32R, dben[:, :])
            # prefetch the first phase-A slabs ahead of everything else
            QH = 4 * H
            QR = 4 * R
            wa_sl = []
            xt_sl = []
            for g in range(IC // 4):
                w4 = wp.tile([P, QH], BF16, tag="wa", bufs=2, name=f"wa4_{g}")
                nc.sync.dma_start(out=w4, in_=wdt_p[:, g * QH:(g + 1) * QH])
                wa_sl.append(w4)
                x4 = wp.tile([P, QR], BF16, tag="xt", bufs=3, name=f"xt4_{g}")
                nc.sync.dma_start(out=x4, in_=xt_p[:, g * QR:(g + 1) * QR])
                xt_sl.append(x4)
            upb_sb = cdma([T, H], F32R, upb[:, :])
            sel_sb = cdma([P, TDC * T], F32R, sel_p[:, :])
            exm_sb = cdma([T, TDC * P], F32R, exm_p[:, :])
            upbt_sb = cdma([P, HC * T], F32R, upbt_p[:, :])
            qb_sb = cdma([P, HC], F32, qb_c[:, :])
            db_sb = cdma([P, TDC], F32, db_c[:, :])
            lng_sb = cdma([P, HC], F32, lng_c[:, :])
            lnb_sb = cdma([P, HC], F32, lnb_c[:, :])
            eps_sb = const.tile([1, 1], F32)
            nc.vector.memset(eps_sb, EPS)

            # input_tensor^T chunks (f32, element-wise use only)
            inpt = []
            for m in range(HC):
                it = acts.tile([P, R], F32, tag="u", bufs=44, name=f"inpt{m}")
                nc.sync.dma_start(out=it, in_=inpt_p[:, m * R:(m + 1) * R])
                inpt.append(it)

            def atile(name, dt=F32R):
                return acts.tile([P, R], dt, tag="u", bufs=44, name=name)

            def wtile5(name):
                return wp.tile([P, TD], F32R, tag="w5", bufs=18, name=name)

            # ═══ Phase A: h^T = dense_w.T.T @ X^T (+ dense_b) ═══
            ht = []       # h^T chunks, f32 (element-wise use)
            prenorm = []  # prenorm^T chunks, f32r
            with tc.tile_pool(name="psA", bufs=1, space="PSUM") as psA:
                psa = [psA.tile([P, R], F32, tag="pa", bufs=8, name=f"psa{m}")
                       for m in range(HC)]
                # rank-1 dense_b via K=1 matmul
                for m in range(HC):
                    nc.tensor.matmul(psa[m][:], dben_sb[:, m * P:(m + 1) * P],
                                     ones_r_sb[:], start=True, stop=False)
                for kc in range(IC):
                    g, kl = kc // 4, kc % 4
                    wsl = wa_sl[g]
                    xsl = xt_sl[g]
                    for m in range(HC):
                        nc.tensor.matmul(
                            psa[m][:], wsl[:, kl * H + m * P:kl * H + (m + 1) * P],
                            xsl[:, kl * R:(kl + 1) * R],
                            start=False, stop=(kc == IC - 1))
                for m in range(HC):
                    h_m = atile(f"ht{m}", F32)
                    nc.scalar.copy(out=h_m[:], in_=psa[m][:])
                    ht.append(h_m)
                    pn = atile(f"prenorm{m}")
                    nc.vector.tensor_tensor(out=pn[:], in0=psa[m][:],
                                            in1=inpt[m][:], op=ALU.add)
                    prenorm.append(pn)

            psM = ctx.enter_context(tc.tile_pool(name="psM", bufs=1, space="PSUM"))
            psX = ctx.enter_context(tc.tile_pool(name="psX", bufs=1, space="PSUM"))
            psS = ctx.enter_context(tc.tile_pool(name="psS", bufs=1, space="PSUM"))

            def pmain(name):
                return psM.tile([P, R], F32, tag="pm", bufs=4, name=name)

            def paux(name):
                return psX.tile([P, R], F32, tag="px", bufs=2, name=name)

            def psmall(name):
                return psS.tile([T, R], F32, tag="ps", bufs=2, name=name)

            # ── LayerNorm helpers (stats over the partition=feature axis) ──
            def ln_stats(chunks, label):
                """-> (mu_b, rstd_b) PSUM [P,R] broadcasts; emits sum MMs now."""
                ssum = psmall(f"{label}_sum")
                ssq = psmall(f"{label}_ssq")
                sqs = []
                for m in range(HC):
                    sq = acts.tile([P, R], F32R, tag="sq", bufs=3,
                                   name=f"{label}_sq{m}")
                    nc.scalar.square(out=sq[:], in_=chunks[m][:].bitcast(F32))
                    sqs.append(sq)
                for m in range(HC):
                    nc.tensor.matmul(ssum[:1, :], ones_p_sb[:], chunks[m][:],
                                     start=(m == 0), stop=(m == HC - 1))
                    nc.tensor.matmul(ssq[:1, :], ones_p_sb[:], sqs[m][:],
                                     start=(m == 0), stop=(m == HC - 1))
                return ssum, ssq

            def ln_finish(ssum, ssq, label):
                """Small-op chain + broadcast matmuls."""
                mu = acts.tile([1, R], F32R, tag="st", bufs=4, name=f"{label}_mu")
                nc.scalar.mul(out=mu[:], in_=ssum[:1, :], mul=1.0 / H)
                msq = acts.tile([1, R], F32, tag="st", bufs=4, name=f"{label}_msq")
                nc.scalar.mul(out=msq[:], in_=ssq[:1, :], mul=1.0 / H)
                musq = acts.tile([1, R], F32, tag="st", bufs=4, name=f"{label}_musq")
                nc.vector.tensor_tensor(out=musq[:], in0=mu[:].bitcast(F32),
                                        in1=mu[:].bitcast(F32), op=ALU.mult)
                var = acts.tile([1, R], F32, tag="st", bufs=4, name=f"{label}_var")
                nc.vector.tensor_tensor(out=var[:], in0=msq[:], in1=musq[:],
                                        op=ALU.subtract)
                sd = acts.tile([1, R], F32, tag="st", bufs=4, name=f"{label}_sd")
                nc.scalar.activation(out=sd[:], in_=var[:], func=AF.Sqrt,
                                     bias=eps_sb[:])
                rstd = acts.tile([1, R], F32R, tag="st", bufs=4, name=f"{label}_rstd")
                with nc.allow_low_precision(reason="f32r feed for broadcast mm"):
                    nc.vector.reciprocal(out=rstd[:], in_=sd[:])
                mu_b = paux(f"{label}_mub")
                nc.tensor.matmul(mu_b[:], ones_r_sb[:, :P], mu[:],
                                 start=True, stop=True)
                rstd_b = paux(f"{label}_rstdb")
                nc.tensor.matmul(rstd_b[:], ones_r_sb[:, :P], rstd[:],
                                 start=True, stop=True)
                return mu_b, rstd_b

            def ln_normalize(chunks, mu_b, rstd_b, g_sb, b_sb, label,
                             out_dt, out_names=None):
                outs = []
                for m in range(HC):
                    t1 = acts.tile([P, R], F32, tag="t1", bufs=3,
                                   name=f"{label}_t1_{m}")
                    nc.vector.tensor_tensor(out=t1[:], in0=chunks[m][:].bitcast(F32),
                                            in1=mu_b[:], op=ALU.subtract)
                    t2 = acts.tile([P, R], F32, tag="t2", bufs=3,
                                   name=f"{label}_t2_{m}")
                    nc.vector.tensor_tensor(out=t2[:], in0=t1[:], in1=rstd_b[:],
                                            op=ALU.mult)
                    name = out_names[m] if out_names else f"{label}_o{m}"
                    o = atile(name, out_dt) if out_names is None else \
                        acts.tile([P, R], out_dt, tag="ov", bufs=3, name=name)
                    nc.scalar.activation(out=o[:], in_=t2[:], func=AF.Identity,
                                         scale=g_sb[:, m:m + 1],
                                         bias=b_sb[:, m:m + 1])
                    outs.append(o)
                return outs

            # ═══ LN1 stats (sums on PE now; finish after C/D) ═══
            ln1_sum, ln1_ssq = ln_stats(prenorm, "ln1")

            # ═══ Phase C: q^T = query_w.T.T @ prenorm^T + query_b ═══
            qw_sl = [wtile5(f"wq{i}") for i in range(2 * HC)]
            for i in range(2 * HC):
                nc.sync.dma_start(out=qw_sl[i], in_=qwt_p[:, i * TD:(i + 1) * TD]
                                  .bitcast(F32R))
            q = []
            for m in range(HC):
                ps = pmain(f"psc{m}")
                for kc in range(HC):
                    nc.tensor.matmul(
                        ps[:],
                        qw_sl[kc * 2 + m // 4][:, (m % 4) * P:(m % 4 + 1) * P],
                        prenorm[kc][:], start=(kc == 0),
                        stop=(kc == HC - 1))
                qm = atile(f"q{m}")
                nc.scalar.activation(out=qm[:], in_=ps[:], func=AF.Identity,
                                     bias=qb_sb[:, m:m + 1])
                q.append(qm)

            # ═══ Phase D: qk^T = key_w.T @ q^T ═══
            kw_sl = [wtile5(f"wk{i}") for i in range(2 * HC)]
            for i in range(2 * HC):
                nc.sync.dma_start(out=kw_sl[i], in_=kw_p[:, i * TD:(i + 1) * TD]
                                  .bitcast(F32R))
            qk = []
            for m in range(HC):
                ps = pmain(f"psd{m}")
                for kc in range(HC):
                    nc.tensor.matmul(
                        ps[:],
                        kw_sl[kc * 2 + m // 4][:, (m % 4) * P:(m % 4 + 1) * P],
                        q[kc][:], start=(kc == 0), stop=(kc == HC - 1))
                qkm = atile(f"qk{m}")
                nc.scalar.copy(out=qkm[:], in_=ps[:])
                qk.append(qkm)

            # ═══ LN1 finish + normalize -> adapter_in^T (f32r) ═══
            mu_b, rstd_b = ln_finish(ln1_sum, ln1_ssq, "ln1")

            # ═══ Phase E: qkproj^T = up_w'(h,(t d)).T @ qk^T ═══
            u1_sl = [wtile5(f"wu1{kc}") for kc in range(HC)]
            for kc in range(HC):
                nc.sync.dma_start(out=u1_sl[kc], in_=uw1_p[:, kc * TD:(kc + 1) * TD]
                                  .bitcast(F32R))
            qkproj = []
            for c in range(TDC):
                ps = pmain(f"pse{c}")
                for kc in range(HC):
                    nc.tensor.matmul(ps[:], u1_sl[kc][:, c * P:(c + 1) * P],
                                     qk[kc][:], start=(kc == 0), stop=(kc == HC - 1))
                qp = atile(f"qkproj{c}", F32)
                nc.scalar.copy(out=qp[:], in_=ps[:])
                qkproj.append(qp)

            ain = ln_normalize(prenorm, mu_b, rstd_b, lng_sb, lnb_sb, "ln1", F32R)

            # ═══ Phase B: down^T = relu(down_w'(h,(t d)).T @ ain^T + down_b) ═══
            dw_sl = [wtile5(f"wd{kc}") for kc in range(HC)]
            for kc in range(HC):
                nc.sync.dma_start(out=dw_sl[kc], in_=dwt_p[:, kc * TD:(kc + 1) * TD]
                                  .bitcast(F32R))
            down = []
            for c in range(TDC):
                ps = pmain(f"psb{c}")
                for kc in range(HC):
                    nc.tensor.matmul(ps[:], dw_sl[kc][:, c * P:(c + 1) * P],
                                     ain[kc][:], start=(kc == 0), stop=(kc == HC - 1))
                dn = atile(f"down{c}")
                nc.scalar.activation(out=dn[:], in_=ps[:], func=AF.Relu,
                                     bias=db_sb[:, c:c + 1])
                down.append(dn)

            # ═══ scores = sum_d down*qkproj + up_b·qk  (PSUM [T, R]) ═══
            scores = psmall("scores")
            for kc in range(HC):
                nc.tensor.matmul(scores[:], upbt_sb[:, kc * T:(kc + 1) * T],
                                 qk[kc][:], start=(kc == 0), stop=False)
            prods = []
            for c in range(TDC):
                pr = atile(f"prod{c}")
                nc.vector.tensor_tensor(out=pr[:], in0=down[c][:].bitcast(F32),
                                        in1=qkproj[c][:], op=ALU.mult)
                prods.append(pr)
            for c in range(TDC):
                nc.tensor.matmul(scores[:], sel_sb[:, c * T:(c + 1) * T],
                                 prods[c][:], start=False, stop=(c == TDC - 1))

            # ═══ softmax over T (partition axis, 8 rows) ═══
            exp_sb = acts.tile([T, R], F32R, tag="s8", bufs=2, name="exp_sb")
            nc.scalar.activation(out=exp_sb[:], in_=scores[:], func=AF.Exp)
            sumexp = psmall("sumexp")
            nc.tensor.matmul(sumexp[:1, :], ones_p_sb[:T, :], exp_sb[:],
                             start=True, stop=True)
            rec = acts.tile([1, R], F32R, tag="st", bufs=4, name="rec")
            with nc.allow_low_precision(reason="softmax recip feeds broadcast mm"):
                nc.vector.reciprocal(out=rec[:], in_=sumexp[:1, :])
            rec8 = psmall("rec8")
            nc.tensor.matmul(rec8[:], ones_r_sb[:, :T], rec[:], start=True, stop=True)
            probs = acts.tile([T, R], F32R, tag="s8", bufs=2, name="probs")
            nc.vector.tensor_tensor(out=probs[:], in0=exp_sb[:].bitcast(F32),
                                    in1=rec8[:], op=ALU.mult)

            # wdown = probs-broadcast * down
            wdown = []
            for c in range(TDC):
                pb = paux(f"pbx{c}")
                nc.tensor.matmul(pb[:], exm_sb[:, c * P:(c + 1) * P], probs[:],
                                 start=True, stop=True)
                wd = atile(f"wdown{c}")
                nc.vector.tensor_tensor(out=wd[:], in0=down[c][:].bitcast(F32),
                                        in1=pb[:], op=ALU.mult)
                wdown.append(wd)

            # ═══ Phase F: mix^T = up_w''((t d),h).T @ wdown + up_b.T @ probs ═══
            u2_sl = [wtile5(f"wu2{i}") for i in range(2 * TDC)]
            for i in range(2 * TDC):
                nc.sync.dma_start(out=u2_sl[i], in_=uw2_p[:, i * TD:(i + 1) * TD]
                                  .bitcast(F32R))
            mixh = []
            for m in range(HC):
                ps = pmain(f"psf{m}")
                nc.tensor.matmul(ps[:], upb_sb[:, m * P:(m + 1) * P], probs[:],
                                 start=True, stop=False)
                for kc in range(TDC):
                    nc.tensor.matmul(
                        ps[:],
                        u2_sl[kc * 2 + m // 4][:, (m % 4) * P:(m % 4 + 1) * P],
                        wdown[kc][:], start=False, stop=(kc == TDC - 1))
                mh = atile(f"mixh{m}")
                nc.vector.tensor_tensor(out=mh[:], in0=ps[:], in1=ht[m][:],
                                        op=ALU.add)
                mixh.append(mh)

            # ═══ Phase G: fusion^T = value_w.T.T @ (mix+h)^T;  pre2 = input + fusion ═══
            vw_sl = [wtile5(f"wv{i}") for i in range(2 * HC)]
            for i in range(2 * HC):
                nc.sync.dma_start(out=vw_sl[i], in_=vwt_p[:, i * TD:(i + 1) * TD]
                                  .bitcast(F32R))
            inpt2 = []
            for m in range(HC):
                it2 = atile(f"inpt2_{m}", F32)
                nc.sync.dma_start(out=it2, in_=inpt_p[:, m * R:(m + 1) * R])
                inpt2.append(it2)
            pre2 = []
            for m in range(HC):
                ps = pmain(f"psg{m}")
                for kc in range(HC):
                    nc.tensor.matmul(
                        ps[:],
                        vw_sl[kc * 2 + m // 4][:, (m % 4) * P:(m % 4 + 1) * P],
                        mixh[kc][:], start=(kc == 0), stop=(kc == HC - 1))
                p2 = atile(f"pre2_{m}")
                nc.vector.tensor_tensor(out=p2[:], in0=ps[:], in1=inpt2[m][:],
                                        op=ALU.add)
                pre2.append(p2)

            # ═══ LN2 -> out ═══
            ln2_sum, ln2_ssq = ln_stats(pre2, "ln2")
            mu2_b, rstd2_b = ln_finish(ln2_sum, ln2_ssq, "ln2")
            outs = ln_normalize(pre2, mu2_b, rstd2_b, lng_sb, lnb_sb, "ln2", F32,
                                out_names=[f"outv{m}" for m in range(HC)])
            for m in range(HC):
                nc.sync.dma_start(out=out_p[:, m * R:(m + 1) * R], in_=outs[m][:])

    nc.compile()
    return nc


_NC_CACHE = None


def _get_nc():
    global _NC_CACHE
    if _NC_CACHE is None:
        _NC_CACHE = build_kernel()
    return _NC_CACHE


def _prep_weights(dense_w, dense_b, ln_g, ln_b, down_w, down_b, up_w, up_b,
                  key_w, key_b, query_w, query_b, value_w):
    f = np.float32
    sel = np.zeros((P, TDC * T), f)
    exm = np.zeros((T, TDC * P), f)
    for c in range(TDC):
        for k in range(P):
            t = c * 2 + (k // 64)
            sel[k, c * T + t] = 1.0
            exm[t, c * P + k] = 1.0
    return {
        "wdt_p": _pack_k(np.ascontiguousarray(dense_w.T).astype(f)).astype(
            ml_dtypes.bfloat16),
        "qwt_p": _pack_k(np.ascontiguousarray(query_w.T).astype(f)),
        "kw_p": _pack_k(np.ascontiguousarray(key_w).astype(f)),
        "vwt_p": _pack_k(np.ascontiguousarray(value_w.T).astype(f)),
        "dwt_p": _pack_k(np.ascontiguousarray(
            down_w.transpose(2, 0, 1).reshape(H, TD)).astype(f)),
        "uw1_p": _pack_k(np.ascontiguousarray(
            up_w.transpose(1, 0, 2).reshape(H, TD)).astype(f)),
        "uw2_p": _pack_k(np.ascontiguousarray(
            up_w.transpose(0, 2, 1).reshape(TD, H)).astype(f)),
        "upb": up_b.astype(f),
        "upbt_p": _pack_k(np.ascontiguousarray(up_b.T).astype(f)),
        "dben": dense_b.reshape(1, H).astype(f),
        "qb_c": np.ascontiguousarray(query_b.reshape(HC, P).T).astype(f),
        "db_c": np.ascontiguousarray(down_b.reshape(TD).reshape(TDC, P).T).astype(f),
        "lng_c": np.ascontiguousarray(ln_g.reshape(HC, P).T).astype(f),
        "lnb_c": np.ascontiguousarray(ln_b.reshape(HC, P).T).astype(f),
        "ones_r": np.ones((1, R), f),
        "ones_p": np.ones((P, 1), f),
        "sel_p": sel,
        "exm_p": exm,
    }


def kernel(hidden_states, input_tensor, dense_w, dense_b, ln_g, ln_b,
           down_w, down_b, up_w, up_b, key_w, key_b, query_w, query_b,
           value_w, _trace=False):
    nc = _get_nc()
    hidden_states = np.asarray(hidden_states, np.float32)
    input_tensor = np.asarray(input_tensor, np.float32)
    wmap = _prep_weights(np.asarray(dense_w), np.asarray(dense_b),
                         np.asarray(ln_g), np.asarray(ln_b),
                         np.asarray(down_w), np.asarray(down_b),
                         np.asarray(up_w), np.asarray(up_b),
                         np.asarray(key_w), np.asarray(key_b),
                         np.asarray(query_w), np.asarray(query_b),
                         np.asarray(value_w))
    # key_b only shifts all task scores equally -> cancels in softmax_t.
    xt = np.ascontiguousarray(hidden_states.reshape(B * S, I).T)   # [I, B*S]
    it = np.ascontiguousarray(input_tensor.reshape(B * S, H).T)    # [H, B*S]
    in_maps = []
    for c in range(NCORES):
        m = dict(wmap)
        m["xt_p"] = _pack_k(np.ascontiguousarray(xt[:, c * R:(c + 1) * R])).astype(
            ml_dtypes.bfloat16)
        m["inpt_p"] = _pack_k(np.ascontiguousarray(it[:, c * R:(c + 1) * R]))
        in_maps.append(m)

    res = run_bass_kernel_spmd(nc, in_maps, core_ids=list(range(NCORES)),
                               trace=_trace)
    out = np.empty((B * S, H), np.float32)
    for c in range(NCORES):
        op = res.results[c]["out_p"]
        oc = op.reshape(P, HC, R).transpose(1, 0, 2).reshape(H, R)
        out[c * R:(c + 1) * R, :] = oc.T
    out = out.reshape(B, S, H)
    if _trace:
        return out, res
    return out


# revision 7
# speedup vs baseline: 1.3023x; 1.0108x over previous
"""Trainium2 Bass kernel for BertOutputWithAdapterFusion (T=8 adapters).

Math (reference):
  h        = hidden_states @ dense_w.T + dense_b          [B,S,H]
  prenorm  = input_tensor + h
  ain      = LN(prenorm)
  down[t]  = relu(ain @ down_w[t].T + down_b[t])          [B,S,D]
  aout[t]  = down[t] @ up_w[t].T + up_b[t]                [B,S,H]
  key[t]   = aout[t] @ key_w.T + key_b
  value[t] = (aout[t] + h) @ value_w.T
  query    = prenorm @ query_w.T + query_b
  scores   = sum(query*key, -1); probs = softmax_t(scores)
  fusion   = sum_t probs[t]*value[t]
  out      = LN(input_tensor + fusion)

Algebraic restructuring used here (exact, up to fp rounding):
  qk       = query @ key_w                   (scores[t] = aout[t]·qk + q·key_b,
                                              and the q·key_b term is constant
                                              over t so it cancels in softmax)
  scores[t]= sum_d down[t]*qkproj[t] + up_b[t]·qk
             where qkproj[t] = qk @ up_w[t]  (aout never materialized)
  fusion   = (mix + h) @ value_w.T
             where mix = sum_t probs[t]*aout[t]
                       = up_w-combine(probs[t]*down[t]) + probs.T @ up_b
                       (uses sum_t probs[t] = 1)
The T=8 adapters are stacked into single [H, T*D] matmuls.

Distribution: data-parallel over the 8 batches (1 batch of 512 rows per
core), all parameters replicated, no collectives.  All on-chip activations
are kept transposed [feature, row] so no on-chip transposes are needed —
every weight is pre-transposed/packed on the host.  Matmuls run as fp32r
(full PE rate at N=512); LayerNorm stats and T-axis softmax reductions run
as ones-vector matmuls over the partition axis.
"""
import sys
import types

sys.path.insert(0, "/opt/trn_rl_repo")

import numpy as np
import ml_dtypes
from contextlib import ExitStack

import concourse.bacc as bacc
import concourse.tile as tile
from concourse import mybir
import concourse.bass_utils as bass_utils
from concourse.bass_utils import run_bass_kernel_spmd

# ── axon NTFF profile hook (image's antenv lacks axon_hooks) ─────────────
try:
    from trn_agent_boot.trn_boot import _ntff_profile_via_ctypes

    _hook = _ntff_profile_via_ctypes("/opt/axon/libaxon_pjrt.so")
except Exception:  # pragma: no cover
    _hook = None
_mod = types.ModuleType("antenv.axon_hooks")
_mod.get_axon_ntff_profile_hook = lambda: _hook
_mod.set_axon_ntff_profile_hook = lambda h: None
sys.modules["antenv.axon_hooks"] = _mod
bass_utils.upload_artifacts = lambda tmpdir: str(tmpdir)

# ── problem shapes (hardcoded per spec) ──────────────────────────────────
B, S, I, H, T, D = 8, 512, 4096, 1024, 8, 64
NCORES = 8
R = (B * S) // NCORES        # 512 rows per core
P = 128
IC, HC = I // P, H // P      # 32, 8
TD = T * D                   # 512
TDC = TD // P                # 4
EPS = 1e-12

F32 = mybir.dt.float32
F32R = mybir.dt.float32r
BF16 = mybir.dt.bfloat16
AF = mybir.ActivationFunctionType
ALU = mybir.AluOpType


def _pack_k(w):
    """[K, M] -> [P, (K//P)*M] so that slab kc (cols kc*M:(kc+1)*M) is the
    [P, M] tile holding rows kc*P..kc*P+P of w, partition-major."""
    K, M = w.shape
    return np.ascontiguousarray(
        w.reshape(K // P, P, M).transpose(1, 0, 2).reshape(P, (K // P) * M)
    )


def build_kernel():
    nc = bacc.Bacc("TRN2", debug=False)

    dram = lambda name, shape: nc.dram_tensor(name, shape, F32, kind="ExternalInput")
    # per-core activations
    xt_p = nc.dram_tensor("xt_p", [P, IC * R], BF16, kind="ExternalInput")
    inpt_p = dram("inpt_p", [P, HC * R])   # packed input_tensor^T slice
    # packed weights (replicated)
    wdt_p = nc.dram_tensor("wdt_p", [P, IC * H], BF16, kind="ExternalInput")
    qwt_p = dram("qwt_p", [P, HC * H])     # query_w.T packed
    kw_p = dram("kw_p", [P, HC * H])       # key_w packed
    vwt_p = dram("vwt_p", [P, HC * H])     # value_w.T packed
    dwt_p = dram("dwt_p", [P, HC * TD])    # down_w 't d h -> h (t d)' packed
    uw1_p = dram("uw1_p", [P, HC * TD])    # up_w 't h d -> h (t d)' packed
    uw2_p = dram("uw2_p", [P, TDC * H])    # up_w 't h d -> (t d) h' packed
    upb = dram("upb", [T, H])              # up_b
    upbt_p = dram("upbt_p", [P, HC * T])   # up_b.T packed
    dben = dram("dben", [1, H])            # dense_b row
    qb_c = dram("qb_c", [P, HC])           # query_b per-chunk cols
    db_c = dram("db_c", [P, TDC])          # down_b per-chunk cols
    lng_c = dram("lng_c", [P, HC])         # ln_g per-chunk cols
    lnb_c = dram("lnb_c", [P, HC])         # ln_b per-chunk cols
    ones_r = dram("ones_r", [1, R])        # ones row
    ones_p = dram("ones_p", [P, 1])        # ones column
    invh_p = dram("invh_p", [P, 1])        # 1/H column (LN mean matmuls)
    sel_p = dram("sel_p", [P, TDC * T])    # per-chunk task-sum selectors
    exm_p = dram("exm_p", [T, TDC * P])    # per-chunk task-broadcast expanders
    out_p = nc.dram_tensor("out_p", [P, HC * R], F32, kind="ExternalOutput")

    with tile.TileContext(nc) as tc:
        with ExitStack() as ctx:
            const = ctx.enter_context(tc.tile_pool(name="const", bufs=1))
            acts = ctx.enter_context(tc.tile_pool(name="acts", bufs=1))
            wp = ctx.enter_context(tc.tile_pool(name="wp", bufs=1))

            def cdma(shape, dt, src):
                t = const.tile(shape, dt, name=f"c_{src.tensor.name}", uniquify=True)
                nc.sync.dma_start(out=t, in_=src if dt is F32 else src.bitcast(F32R))
                return t

            ones_r_sb = cdma([1, R], F32R, ones_r[:, :])
            ones_p_sb = cdma([P, 1], F32R, ones_p[:, :])
            invh_sb = cdma([P, 1], F32R, invh_p[:, :])
            dben_sb = cdma([1, H], F32R, dben[:, :])
            # prefetch the first phase-A slabs ahead of everything else
            QH = 4 * H
            QR = 4 * R
            wa_sl = []
            xt_sl = []
            for g in range(IC // 4):
                w4 = wp.tile([P, QH], BF16, tag="wa", bufs=2, name=f"wa4_{g}")
                nc.sync.dma_start(out=w4, in_=wdt_p[:, g * QH:(g + 1) * QH])
                wa_sl.append(w4)
                x4 = wp.tile([P, QR], BF16, tag="xt", bufs=3, name=f"xt4_{g}")
                nc.sync.dma_start(out=x4, in_=xt_p[:, g * QR:(g + 1) * QR])
                xt_sl.append(x4)
            upb_sb = cdma([T, H], F32R, upb[:, :])
            sel_sb = cdma([P, TDC * T], F32R, sel_p[:, :])
            exm_sb = cdma([T, TDC * P], F32R, exm_p[:, :])
            upbt_sb = cdma([P, HC * T], F32R, upbt_p[:, :])
            qb_sb = cdma([P, HC], F32, qb_c[:, :])
            db_sb = cdma([P, TDC], F32, db_c[:, :])
            lng_sb = cdma([P, HC], F32, lng_c[:, :])
            lnb_sb = cdma([P, HC], F32, lnb_c[:, :])
            eps_sb = const.tile([1, 1], F32)
            nc.vector.memset(eps_sb, EPS)

            # input_tensor^T chunks (f32, element-wise use only)
            inpt = []
            for m in range(HC):
                it = acts.tile([P, R], F32, tag="u", bufs=43, name=f"inpt{m}")
                nc.sync.dma_start(out=it, in_=inpt_p[:, m * R:(m + 1) * R])
                inpt.append(it)

            def atile(name, dt=F32R):
                return acts.tile([P, R], dt, tag="u", bufs=43, name=name)

            def wtile5(name):
                return wp.tile([P, TD], F32R, tag="w5", bufs=18, name=name)

            # ═══ Phase A: h^T = dense_w.T.T @ X^T (+ dense_b) ═══
            ht = []       # h^T chunks, f32 (element-wise use)
            prenorm = []  # prenorm^T chunks, f32r
            with tc.tile_pool(name="psA", bufs=1, space="PSUM") as psA:
                psa = [psA.tile([P, R], F32, tag="pa", bufs=8, name=f"psa{m}")
                       for m in range(HC)]
                # rank-1 dense_b via K=1 matmul
                for m in range(HC):
                    nc.tensor.matmul(psa[m][:], dben_sb[:, m * P:(m + 1) * P],
                                     ones_r_sb[:], start=True, stop=False)
                NG = IC // 4
                for g in range(NG - 1):
                    wsl, xsl = wa_sl[g], xt_sl[g]
                    for kl in range(4):
                        for m in range(HC):
                            nc.tensor.matmul(
                                psa[m][:],
                                wsl[:, kl * H + m * P:kl * H + (m + 1) * P],
                                xsl[:, kl * R:(kl + 1) * R],
                                start=False, stop=False)
                # last group m-outer: each m finishes early so its eviction
                # overlaps the remaining matmuls
                wsl, xsl = wa_sl[NG - 1], xt_sl[NG - 1]
                for m in range(HC):
                    for kl in range(4):
                        nc.tensor.matmul(
                            psa[m][:],
                            wsl[:, kl * H + m * P:kl * H + (m + 1) * P],
                            xsl[:, kl * R:(kl + 1) * R],
                            start=False, stop=(kl == 3))
                    h_m = atile(f"ht{m}", F32)
                    nc.scalar.copy(out=h_m[:], in_=psa[m][:])
                    ht.append(h_m)
                    pn = atile(f"prenorm{m}")
                    nc.vector.tensor_tensor(out=pn[:], in0=psa[m][:],
                                            in1=inpt[m][:], op=ALU.add)
                    prenorm.append(pn)

            psM = ctx.enter_context(tc.tile_pool(name="psM", bufs=1, space="PSUM"))
            psX = ctx.enter_context(tc.tile_pool(name="psX", bufs=1, space="PSUM"))
            psS = ctx.enter_context(tc.tile_pool(name="psS", bufs=1, space="PSUM"))

            def pmain(name):
                return psM.tile([P, R], F32, tag="pm", bufs=4, name=name)

            def paux(name):
                return psX.tile([P, R], F32, tag="px", bufs=2, name=name)

            def psmall(name):
                return psS.tile([T, R], F32, tag="ps", bufs=2, name=name)

            # ── LayerNorm helpers (stats over the partition=feature axis) ──
            def ln_stats(chunks, label):
                """-> (mu_b, rstd_b) PSUM [P,R] broadcasts; emits sum MMs now."""
                ssum = psmall(f"{label}_sum")
                ssq = psmall(f"{label}_ssq")
                sqs = []
                for m in range(HC):
                    sq = acts.tile([P, R], F32R, tag="sq", bufs=3,
                                   name=f"{label}_sq{m}")
                    nc.scalar.square(out=sq[:], in_=chunks[m][:].bitcast(F32))
                    sqs.append(sq)
                for m in range(HC):
                    nc.tensor.matmul(ssum[:1, :], invh_sb[:], chunks[m][:],
                                     start=(m == 0), stop=(m == HC - 1))
                    nc.tensor.matmul(ssq[:1, :], invh_sb[:], sqs[m][:],
                                     start=(m == 0), stop=(m == HC - 1))
                return ssum, ssq

            def ln_finish(ssum, ssq, label):
                """Small-op chain + broadcast matmuls; broadcasts land in SBUF
                so the normalize DVE ops run in 2x (SBUF-only) mode."""
                mu = acts.tile([1, R], F32R, tag="st", bufs=4, name=f"{label}_mu")
                nc.scalar.copy(out=mu[:], in_=ssum[:1, :])
                musq = acts.tile([1, R], F32, tag="st", bufs=4, name=f"{label}_musq")
                nc.scalar.square(out=musq[:], in_=ssum[:1, :])
                var = acts.tile([1, R], F32, tag="st", bufs=4, name=f"{label}_var")
                nc.vector.tensor_tensor(out=var[:], in0=ssq[:1, :], in1=musq[:],
                                        op=ALU.subtract)
                sd = acts.tile([1, R], F32, tag="st", bufs=4, name=f"{label}_sd")
                nc.scalar.activation(out=sd[:], in_=var[:], func=AF.Sqrt,
                                     bias=eps_sb[:])
                rstd = acts.tile([1, R], F32R, tag="st", bufs=4, name=f"{label}_rstd")
                with nc.allow_low_precision(reason="f32r feed for broadcast mm"):
                    nc.vector.reciprocal(out=rstd[:], in_=sd[:])
                mu_b = paux(f"{label}_mub")
                nc.tensor.matmul(mu_b[:], ones_r_sb[:, :P], mu[:],
                                 start=True, stop=True)
                rstd_b = paux(f"{label}_rstdb")
                nc.tensor.matmul(rstd_b[:], ones_r_sb[:, :P], rstd[:],
                                 start=True, stop=True)
                mu_bs = acts.tile([P, R], F32, tag="bc", bufs=3,
                                  name=f"{label}_mubs")
                nc.scalar.copy(out=mu_bs[:], in_=mu_b[:])
                rstd_bs = acts.tile([P, R], F32, tag="bc", bufs=3,
                                    name=f"{label}_rstdbs")
                nc.scalar.copy(out=rstd_bs[:], in_=rstd_b[:])
                return mu_bs, rstd_bs

            def ln_normalize(chunks, mu_b, rstd_b, g_sb, b_sb, label,
                             out_dt, out_names=None):
                outs = []
                for m in range(HC):
                    t1 = acts.tile([P, R], F32, tag="t1", bufs=3,
                                   name=f"{label}_t1_{m}")
                    nc.vector.tensor_tensor(out=t1[:], in0=chunks[m][:].bitcast(F32),
                                            in1=mu_b[:], op=ALU.subtract)
                    t2 = acts.tile([P, R], F32, tag="t2", bufs=3,
                                   name=f"{label}_t2_{m}")
                    nc.vector.tensor_tensor(out=t2[:], in0=t1[:], in1=rstd_b[:],
                                            op=ALU.mult)
                    name = out_names[m] if out_names else f"{label}_o{m}"
                    o = atile(name, out_dt) if out_names is None else \
                        acts.tile([P, R], out_dt, tag="ov", bufs=3, name=name)
                    nc.scalar.activation(out=o[:], in_=t2[:], func=AF.Identity,
                                         scale=g_sb[:, m:m + 1],
                                         bias=b_sb[:, m:m + 1])
                    outs.append(o)
                return outs

            # ═══ LN1 stats (sums on PE now; finish after C/D) ═══
            ln1_sum, ln1_ssq = ln_stats(prenorm, "ln1")

            # ═══ Phase C: q^T = query_w.T.T @ prenorm^T + query_b ═══
            qw_sl = [wtile5(f"wq{i}") for i in range(2 * HC)]
            for i in range(2 * HC):
                nc.sync.dma_start(out=qw_sl[i], in_=qwt_p[:, i * TD:(i + 1) * TD]
                                  .bitcast(F32R))
            q = []
            for m in range(HC):
                ps = pmain(f"psc{m}")
                for kc in range(HC):
                    nc.tensor.matmul(
                        ps[:],
                        qw_sl[kc * 2 + m // 4][:, (m % 4) * P:(m % 4 + 1) * P],
                        prenorm[kc][:], start=(kc == 0),
                        stop=(kc == HC - 1))
                qm = atile(f"q{m}")
                nc.scalar.activation(out=qm[:], in_=ps[:], func=AF.Identity,
                                     bias=qb_sb[:, m:m + 1])
                q.append(qm)

            # ═══ Phase D: qk^T = key_w.T @ q^T ═══
            kw_sl = [wtile5(f"wk{i}") for i in range(2 * HC)]
            for i in range(2 * HC):
                nc.sync.dma_start(out=kw_sl[i], in_=kw_p[:, i * TD:(i + 1) * TD]
                                  .bitcast(F32R))
            qk = []
            for m in range(HC):
                ps = pmain(f"psd{m}")
                for kc in range(HC):
                    nc.tensor.matmul(
                        ps[:],
                        kw_sl[kc * 2 + m // 4][:, (m % 4) * P:(m % 4 + 1) * P],
                        q[kc][:], start=(kc == 0), stop=(kc == HC - 1))
                qkm = atile(f"qk{m}")
                nc.scalar.copy(out=qkm[:], in_=ps[:])
                qk.append(qkm)

            # ═══ LN1 finish + normalize -> adapter_in^T (f32r) ═══
            mu_b, rstd_b = ln_finish(ln1_sum, ln1_ssq, "ln1")

            # ═══ Phase E: qkproj^T = up_w'(h,(t d)).T @ qk^T ═══
            u1_sl = [wtile5(f"wu1{kc}") for kc in range(HC)]
            for kc in range(HC):
                nc.sync.dma_start(out=u1_sl[kc], in_=uw1_p[:, kc * TD:(kc + 1) * TD]
                                  .bitcast(F32R))
            qkproj = []
            for c in range(TDC):
                ps = pmain(f"pse{c}")
                for kc in range(HC):
                    nc.tensor.matmul(ps[:], u1_sl[kc][:, c * P:(c + 1) * P],
                                     qk[kc][:], start=(kc == 0), stop=(kc == HC - 1))
                qp = atile(f"qkproj{c}", F32)
                nc.scalar.copy(out=qp[:], in_=ps[:])
                qkproj.append(qp)

            ain = ln_normalize(prenorm, mu_b, rstd_b, lng_sb, lnb_sb, "ln1", F32R)

            # ═══ Phase B: down^T = relu(down_w'(h,(t d)).T @ ain^T + down_b) ═══
            dw_sl = [wtile5(f"wd{kc}") for kc in range(HC)]
            for kc in range(HC):
                nc.sync.dma_start(out=dw_sl[kc], in_=dwt_p[:, kc * TD:(kc + 1) * TD]
                                  .bitcast(F32R))
            down = []
            for c in range(TDC):
                ps = pmain(f"psb{c}")
                for kc in range(HC):
                    nc.tensor.matmul(ps[:], dw_sl[kc][:, c * P:(c + 1) * P],
                                     ain[kc][:], start=(kc == 0), stop=(kc == HC - 1))
                dn = atile(f"down{c}")
                nc.scalar.activation(out=dn[:], in_=ps[:], func=AF.Relu,
                                     bias=db_sb[:, c:c + 1])
                down.append(dn)

            # ═══ scores = sum_d down*qkproj + up_b·qk  (PSUM [T, R]) ═══
            scores = psmall("scores")
            for kc in range(HC):
                nc.tensor.matmul(scores[:], upbt_sb[:, kc * T:(kc + 1) * T],
                                 qk[kc][:], start=(kc == 0), stop=False)
            prods = []
            for c in range(TDC):
                pr = atile(f"prod{c}")
                nc.vector.tensor_tensor(out=pr[:], in0=down[c][:].bitcast(F32),
                                        in1=qkproj[c][:], op=ALU.mult)
                prods.append(pr)
            for c in range(TDC):
                nc.tensor.matmul(scores[:], sel_sb[:, c * T:(c + 1) * T],
                                 prods[c][:], start=False, stop=(c == TDC - 1))

            # ═══ softmax over T (partition axis, 8 rows) ═══
            # exp weights are used UNNORMALIZED; the 1/sum(exp) factor is
            # applied per-column at the mix eviction (it distributes through
            # the linear combine), keeping it off the PE critical path.
            exp_sb = acts.tile([T, R], F32R, tag="s8", bufs=2, name="exp_sb")
            nc.scalar.activation(out=exp_sb[:], in_=scores[:], func=AF.Exp)
            wdown = []
            for c in range(TDC):
                pb = paux(f"pbx{c}")
                nc.tensor.matmul(pb[:], exm_sb[:, c * P:(c + 1) * P], exp_sb[:],
                                 start=True, stop=True)
                wd = atile(f"wdown{c}")
                nc.vector.tensor_tensor(out=wd[:], in0=down[c][:].bitcast(F32),
                                        in1=pb[:], op=ALU.mult)
                wdown.append(wd)
            sumexp = psmall("sumexp")
            nc.tensor.matmul(sumexp[:1, :], ones_p_sb[:T, :], exp_sb[:],
                             start=True, stop=True)
            rec = acts.tile([1, R], F32R, tag="st", bufs=4, name="rec")
            with nc.allow_low_precision(reason="softmax recip feeds broadcast mm"):
                nc.vector.reciprocal(out=rec[:], in_=sumexp[:1, :])
            recb = paux("recb")
            nc.tensor.matmul(recb[:], ones_r_sb[:, :P], rec[:],
                             start=True, stop=True)
            recb_sb = acts.tile([P, R], F32, tag="bc", bufs=3, name="recb_sb")
            nc.scalar.copy(out=recb_sb[:], in_=recb[:])

            # ═══ Phase F: mix^T = up_w''((t d),h).T @ wdown + up_b.T @ probs ═══
            u2_sl = [wtile5(f"wu2{i}") for i in range(2 * TDC)]
            for i in range(2 * TDC):
                nc.sync.dma_start(out=u2_sl[i], in_=uw2_p[:, i * TD:(i + 1) * TD]
                                  .bitcast(F32R))
            mixh = []
            for m in range(HC):
                ps = pmain(f"psf{m}")
                nc.tensor.matmul(ps[:], upb_sb[:, m * P:(m + 1) * P], exp_sb[:],
                                 start=True, stop=False)
                for kc in range(TDC):
                    nc.tensor.matmul(
                        ps[:],
                        u2_sl[kc * 2 + m // 4][:, (m % 4) * P:(m % 4 + 1) * P],
                        wdown[kc][:], start=False, stop=(kc == TDC - 1))
                msc = acts.tile([P, R], F32, tag="t1", bufs=3, name=f"msc{m}")
                nc.vector.tensor_tensor(out=msc[:], in0=ps[:], in1=recb_sb[:],
                                        op=ALU.mult)
                mh = atile(f"mixh{m}")
                nc.vector.tensor_tensor(out=mh[:], in0=msc[:], in1=ht[m][:],
                                        op=ALU.add)
                mixh.append(mh)

            # ═══ Phase G: fusion^T = value_w.T.T @ (mix+h)^T;  pre2 = input + fusion ═══
            vw_sl = [wtile5(f"wv{i}") for i in range(2 * HC)]
            for i in range(2 * HC):
                nc.sync.dma_start(out=vw_sl[i], in_=vwt_p[:, i * TD:(i + 1) * TD]
                                  .bitcast(F32R))
            inpt2 = []
            for m in range(HC):
                it2 = atile(f"inpt2_{m}", F32)
                nc.sync.dma_start(out=it2, in_=inpt_p[:, m * R:(m + 1) * R])
                inpt2.append(it2)
            pre2 = []
            for m in range(HC):
                ps = pmain(f"psg{m}")
                for kc in range(HC):
                    nc.tensor.matmul(
                        ps[:],
                        vw_sl[kc * 2 + m // 4][:, (m % 4) * P:(m % 4 + 1) * P],
                        mixh[kc][:], start=(kc == 0), stop=(kc == HC - 1))
                p2 = atile(f"pre2_{m}")
                nc.vector.tensor_tensor(out=p2[:], in0=ps[:], in1=inpt2[m][:],
                                        op=ALU.add)
                pre2.append(p2)

            # ═══ LN2 -> out ═══
            ln2_sum, ln2_ssq = ln_stats(pre2, "ln2")
            mu2_b, rstd2_b = ln_finish(ln2_sum, ln2_ssq, "ln2")
            outs = ln_normalize(pre2, mu2_b, rstd2_b, lng_sb, lnb_sb, "ln2", F32,
                                out_names=[f"outv{m}" for m in range(HC)])
            for m in range(HC):
                nc.sync.dma_start(out=out_p[:, m * R:(m + 1) * R], in_=outs[m][:])

    nc.compile()
    return nc


_NC_CACHE = None


def _get_nc():
    global _NC_CACHE
    if _NC_CACHE is None:
        _NC_CACHE = build_kernel()
    return _NC_CACHE


def _prep_weights(dense_w, dense_b, ln_g, ln_b, down_w, down_b, up_w, up_b,
                  key_w, key_b, query_w, query_b, value_w):
    f = np.float32
    sel = np.zeros((P, TDC * T), f)
    exm = np.zeros((T, TDC * P), f)
    for c in range(TDC):
        for k in range(P):
            t = c * 2 + (k // 64)
            sel[k, c * T + t] = 1.0
            exm[t, c * P + k] = 1.0
    return {
        "wdt_p": _pack_k(np.ascontiguousarray(dense_w.T).astype(f)).astype(
            ml_dtypes.bfloat16),
        "qwt_p": _pack_k(np.ascontiguousarray(query_w.T).astype(f)),
        "kw_p": _pack_k(np.ascontiguousarray(key_w).astype(f)),
        "vwt_p": _pack_k(np.ascontiguousarray(value_w.T).astype(f)),
        "dwt_p": _pack_k(np.ascontiguousarray(
            down_w.transpose(2, 0, 1).reshape(H, TD)).astype(f)),
        "uw1_p": _pack_k(np.ascontiguousarray(
            up_w.transpose(1, 0, 2).reshape(H, TD)).astype(f)),
        "uw2_p": _pack_k(np.ascontiguousarray(
            up_w.transpose(0, 2, 1).reshape(TD, H)).astype(f)),
        "upb": up_b.astype(f),
        "upbt_p": _pack_k(np.ascontiguousarray(up_b.T).astype(f)),
        "dben": dense_b.reshape(1, H).astype(f),
        "qb_c": np.ascontiguousarray(query_b.reshape(HC, P).T).astype(f),
        "db_c": np.ascontiguousarray(down_b.reshape(TD).reshape(TDC, P).T).astype(f),
        "lng_c": np.ascontiguousarray(ln_g.reshape(HC, P).T).astype(f),
        "lnb_c": np.ascontiguousarray(ln_b.reshape(HC, P).T).astype(f),
        "ones_r": np.ones((1, R), f),
        "ones_p": np.ones((P, 1), f),
        "invh_p": np.full((P, 1), 1.0 / H, f),
        "sel_p": sel,
        "exm_p": exm,
    }


def kernel(hidden_states, input_tensor, dense_w, dense_b, ln_g, ln_b,
           down_w, down_b, up_w, up_b, key_w, key_b, query_w, query_b,
           value_w, _trace=False):
    nc = _get_nc()
    hidden_states = np.asarray(hidden_states, np.float32)
    input_tensor = np.asarray(input_tensor, np.float32)
    wmap = _prep_weights(np.asarray(dense_w), np.asarray(dense_b),
                         np.asarray(ln_g), np.asarray(ln_b),
                         np.asarray(down_w), np.asarray(down_b),
                         np.asarray(up_w), np.asarray(up_b),
                         np.asarray(key_w), np.asarray(key_b),
                         np.asarray(query_w), np.asarray(query_b),
                         np.asarray(value_w))
    # key_b only shifts all task scores equally -> cancels in softmax_t.
    xt = np.ascontiguousarray(hidden_states.reshape(B * S, I).T)   # [I, B*S]
    it = np.ascontiguousarray(input_tensor.reshape(B * S, H).T)    # [H, B*S]
    in_maps = []
    for c in range(NCORES):
        m = dict(wmap)
        m["xt_p"] = _pack_k(np.ascontiguousarray(xt[:, c * R:(c + 1) * R])).astype(
            ml_dtypes.bfloat16)
        m["inpt_p"] = _pack_k(np.ascontiguousarray(it[:, c * R:(c + 1) * R]))
        in_maps.append(m)

    res = run_bass_kernel_spmd(nc, in_maps, core_ids=list(range(NCORES)),
                               trace=_trace)
    out = np.empty((B * S, H), np.float32)
    for c in range(NCORES):
        op = res.results[c]["out_p"]
        oc = op.reshape(P, HC, R).transpose(1, 0, 2).reshape(H, R)
        out[c * R:(c + 1) * R, :] = oc.T
    out = out.reshape(B, S, H)
    if _trace:
        return out, res
    return out
